# revision 1
# baseline (speedup 1.0000x reference)
"""GAT-style attention conv (nn_GatConv_35192962024014) on 8 NeuronCores.

Sharding: batch dim B=256 split 8 ways (32 sessions/core), attention
params A (4,100) replicated. No cross-device communication needed.

Math (matches reference):
  e[k,b,i,j] = leaky_relu(sum_d h[b,i,d] h[b,j,d] A[k,d], 0.2)
  alpha[b,i,j] = e[adj[b,i,j]-1, b, i, j] if adj in 1..4 else -9e15
  out = softmax(alpha, axis=-1) @ h
"""
import numpy as np
import jax
import jax.numpy as jnp

NEG_INF = -9e15
LEAKY_SLOPE = 0.2
N_CORES = 8
B, N, E = 256, 300, 100


def _per_core(h, adj, A):
    # h: [b, N, E] f32; adj: [b, N, N] int32; A: [4, E] f32
    # hA[k,b,i,d] = h[b,i,d] * A[k,d]; e[k,b,i,j] = hA[k,b,i,:] @ h[b,j,:]
    hA = h[None, :, :, :] * A[:, None, None, :]          # [4, b, N, E]
    e = jnp.einsum('kbid,bjd->kbij', hA, h)              # [4, b, N, N]
    e = jnp.where(e > 0, e, LEAKY_SLOPE * e)
    alpha = jnp.full(adj.shape, NEG_INF, dtype=jnp.float32)
    for k in range(4):
        alpha = jnp.where(adj == k + 1, e[k], alpha)
    alpha = jax.nn.softmax(alpha, axis=-1)
    return jnp.matmul(alpha, h)                          # [b, N, E]


_pmapped = jax.pmap(_per_core, in_axes=(0, 0, None))


def kernel(item_embeddings: np.ndarray, adj: np.ndarray, A: np.ndarray) -> np.ndarray:
    h = np.asarray(item_embeddings, dtype=np.float32).reshape(N_CORES, B // N_CORES, N, E)
    # int64 unsupported on device; values are 0..4 so int8 is lossless
    # and cuts the dominant 184MB host->device transfer by 8x
    a32 = np.asarray(adj).astype(np.int8).reshape(N_CORES, B // N_CORES, N, N)
    Af = np.asarray(A, dtype=np.float32)
    out = _pmapped(h, a32, Af)
    out = np.asarray(jax.device_get(out)).reshape(B, N, E).astype(np.float32)
    return out

